# revision 1
# baseline (speedup 1.0000x reference)
"""Trainium2 Bass kernel for nn_MultiHeadAttention (B=4, S=2048, D=1024, H=16, HD=64).

Sharding: 8 cores = 4 batches (data parallel) x 2 head-groups of 8 heads
(tensor parallel). Each core computes its batch's QKV projections for its 8
heads, full softmax attention, and the partial output projection for its head
group. The host sums the two head-group partials per batch (the hinted
all-reduce, done at gather time) and adds the output bias.

Measured-on-HW design notes (vs the 747us fp32r baseline -> ~601us):
  - All matmul operands bf16 (host converts): fp32r tripped the HAM activity
    throttle to a 4/8 duty cycle for most of the sweep; bf16 reduces it and
    halves DMA + SBUF. PSUM stays fp32. rel err ~5e-3 (gate is 2e-2).
  - The per-unit scores/exp phase (ACT-paced) and PV phase (PE-dense)
    deliberately ALTERNATE. Interleaving them (PV trailing the score groups)
    measured WORSE: concurrent PE+ACT load raises the HAM duty-cycle tax more
    than the overlap saves. The chip power cap, not engine occupancy, is the
    binding constraint.
  - exp on ACT in [128,1024] instructions (per-instruction overhead ~270ns
    makes [128,512] exps a big regression), written as bf16 E^T tiles.
  - 1/Z via Ln+Exp(-x) on ACT (shared table set, no reloads); the exact DVE
    reciprocal is 3.3us/call and the approx custom-DVE op returns garbage on
    HW (uop table not honored by this runtime path).
  - Pair-outer sweep; pair p+1's Q/K projection quarters are emitted between
    units of pair p; out-proj per qc after pair 3's unit.
  - x^T loaded column-major in [128,512] pieces across 3 DMA queues
    (sync/scalar/gpsimd) so the first projection chain starts ~6us earlier.
  - PV: lhsT = V' [128, 65] with a ones column collecting Z in psum row 64
    (M=65 wastes half the PE array but every Z alternative costs a full
    N=512 stream or >128 output partitions - verified exhaustively).
PSUM budget: pp 2x[128,512] + pssc 2x[128,1024] + pso 2x[65,512] = 8 banks.
"""

import numpy as np
import ml_dtypes
from contextlib import ExitStack

B, S, D = 4, 2048, 1024
H, HD = 16, 64
NCORES = 8
HPC = H // 2            # heads per core = 8
PAIRS = HPC // 2        # head pairs per core = 4
DH = HPC * HD           # per-core head dims = 512
P = 128
TOK_T = S // P          # 16 token tiles of 128
QCC = S // 512          # 4 query chunks of 512
QW = 512
KC = S // P             # 16 key chunks of 128
FC = D // P             # 8 feature chunks of 128
KPG = 2                 # key tiles per exp group
NG = KC // KPG          # 8 exp groups per (pair, qc)

_CACHE = {}


def _build():
    import concourse.bacc as bacc
    import concourse.mybir as mybir
    import concourse.tile as tile

    dt = mybir.dt
    f32 = dt.float32
    bf16 = dt.bfloat16
    AF = mybir.ActivationFunctionType

    nc = bacc.Bacc("TRN2", target_bir_lowering=False, debug=False)

    xqT = nc.dram_tensor("xqT", [D, S], bf16, kind="ExternalInput")
    xkT = nc.dram_tensor("xkT", [D, S], bf16, kind="ExternalInput")
    xvT = nc.dram_tensor("xvT", [D, S], bf16, kind="ExternalInput")
    wq = nc.dram_tensor("wq", [D, DH], bf16, kind="ExternalInput")
    wk = nc.dram_tensor("wk", [D, DH], bf16, kind="ExternalInput")
    wv = nc.dram_tensor("wv", [D, DH], bf16, kind="ExternalInput")
    wo = nc.dram_tensor("wo", [DH, D], bf16, kind="ExternalInput")
    biases = nc.dram_tensor("biases", [P, 3 * PAIRS], f32, kind="ExternalInput")
    out = nc.dram_tensor("out", [S, D], f32, kind="ExternalOutput")

    SCALE = 1.0 / float(np.sqrt(HD))

    def mmr(psum, lhsT, rhs, **kw):
        nc.tensor.matmul(psum, lhsT, rhs, **kw)

    with tile.TileContext(nc, pool_alloc_mode="queue") as tc, ExitStack() as ctx:
        # ---- pools ----
        xq_pool = ctx.enter_context(tc.tile_pool(name="xq", bufs=FC))
        xk_pool = ctx.enter_context(tc.tile_pool(name="xk", bufs=FC))
        wqk_pool = ctx.enter_context(tc.tile_pool(name="wqk", bufs=2 * FC))
        wo_pool = ctx.enter_context(tc.tile_pool(name="wop", bufs=2 * PAIRS))
        qt_pool = ctx.enter_context(tc.tile_pool(name="qt", bufs=PAIRS))
        kt_pool = ctx.enter_context(tc.tile_pool(name="kt", bufs=PAIRS))
        vpr_pool = ctx.enter_context(tc.tile_pool(name="vpr", bufs=TOK_T))
        ot_pool = ctx.enter_context(tc.tile_pool(name="ot", bufs=16))
        zr_pool = ctx.enter_context(tc.tile_pool(name="zr", bufs=2))
        zb_pool = ctx.enter_context(tc.tile_pool(name="zb", bufs=2))
        os_pool = ctx.enter_context(tc.tile_pool(name="os", bufs=2))
        bias_pool = ctx.enter_context(tc.tile_pool(name="bias", bufs=1))

        # ---- DMA: spread input loads across engine queues ----
        bias_t = bias_pool.tile([P, 3 * PAIRS], f32, name="bias", tag="bias")
        nc.sync.dma_start(bias_t[:], biases[:])
        bq_t = {p: bias_t[:, p:p + 1] for p in range(PAIRS)}
        bk_t = {p: bias_t[:, PAIRS + p:PAIRS + p + 1] for p in range(PAIRS)}
        bv_t = {p: bias_t[:, 2 * PAIRS + p:2 * PAIRS + p + 1] for p in range(PAIRS)}

        vctx = ExitStack()
        xv_pool = vctx.enter_context(tc.tile_pool(name="xv", bufs=FC))
        wv_pool = vctx.enter_context(tc.tile_pool(name="wvp", bufs=FC))
        pp = ctx.enter_context(tc.tile_pool(name="pp", bufs=2, space="PSUM"))

        wq_t, wk_t, wv_t = {}, {}, {}
        xq_t, xk_t, xv_t = [], [], []
        for f in range(FC):
            t = wqk_pool.tile([P, DH], bf16, name=f"wq_{f}", tag="wqk")
            nc.sync.dma_start(t[:], wq[f * P:(f + 1) * P, :])
            wq_t[f] = t
            t = wqk_pool.tile([P, DH], bf16, name=f"wk_{f}", tag="wqk")
            nc.scalar.dma_start(t[:], wk[f * P:(f + 1) * P, :])
            wk_t[f] = t
            t = wv_pool.tile([P, DH], bf16, name=f"wv_{f}", tag="wvp")
            nc.gpsimd.dma_start(t[:], wv[f * P:(f + 1) * P, :])
            wv_t[f] = t
        for f in range(FC):
            xq_t.append(xq_pool.tile([P, S], bf16, name=f"xq_{f}", tag="xq"))
            xk_t.append(xk_pool.tile([P, S], bf16, name=f"xk_{f}", tag="xk"))
            xv_t.append(xv_pool.tile([P, S], bf16, name=f"xv_{f}", tag="xv"))
        for cg in range(QCC):
            cs = slice(cg * QW, (cg + 1) * QW)
            for f in range(FC):
                nc.sync.dma_start(xq_t[f][:, cs], xqT[f * P:(f + 1) * P, cs])
                nc.scalar.dma_start(xk_t[f][:, cs], xkT[f * P:(f + 1) * P, cs])
                nc.gpsimd.dma_start(xv_t[f][:, cs], xvT[f * P:(f + 1) * P, cs])
        wo_t = {}
        for p in range(PAIRS):
            for dc in range(2):
                t = wo_pool.tile([P, QW], bf16, name=f"wo_{p}_{dc}", tag="wop")
                nc.scalar.dma_start(t[:], wo[p * P:(p + 1) * P,
                                            dc * QW:(dc + 1) * QW])
                wo_t[(p, dc)] = t

        qt_t = [qt_pool.tile([P, S], bf16, name=f"qt_{p}", tag="qt")
                for p in range(PAIRS)]
        kt_t = [kt_pool.tile([P, S], bf16, name=f"kt_{p}", tag="kt")
                for p in range(PAIRS)]

        # ---- Q/K projection for one pair, one query-chunk quarter ----
        def qk_proj_quarter(p, tc4, pool):
            for (x_t, w_t, dst, b_t, nm) in ((xq_t, wq_t, qt_t, bq_t, "q"),
                                             (xk_t, wk_t, kt_t, bk_t, "k")):
                ps = pool.tile([P, QW], f32, name=f"ps{nm}_{p}_{tc4}", tag=pool.name)
                for f in range(FC):
                    mmr(ps[:], w_t[f][:, p * P:(p + 1) * P],
                        x_t[f][:, tc4 * QW:(tc4 + 1) * QW],
                        start=(f == 0), stop=(f == FC - 1))
                nc.vector.tensor_scalar_add(
                    dst[p][:, tc4 * QW:(tc4 + 1) * QW], ps[:], b_t[p][:])

        # ---- Phase A: Q/K projection for pair 0 ----
        for tc4 in range(QCC):
            qk_proj_quarter(0, tc4, pp)

        # ---- Phase B: V projection into resident V' tiles ----
        vpr_t = []
        for tci in range(TOK_T):
            ps = pp.tile([P, DH], f32, name=f"psv_{tci}", tag="pp")
            for f in range(FC):
                mmr(ps[:], xv_t[f][:, tci * P:(tci + 1) * P], wv_t[f][:],
                    start=(f == 0), stop=(f == FC - 1))
            vt = vpr_pool.tile([P, HPC * (HD + 1)], bf16,
                               name=f"vpr_{tci}", tag="vpr")
            v3 = vt.rearrange("p (h c) -> p h c", c=HD + 1)
            nc.gpsimd.memset(v3[:, :, HD:HD + 1], 1.0)
            nc.vector.tensor_copy(v3[:, :, 0:HD],
                                  ps.rearrange("p (h c) -> p h c", c=HD))
            vpr_t.append(vt)
        vctx.close()  # xv / wv SBUF freed for the E^T pool
        et_pool = ctx.enter_context(tc.tile_pool(name="et", bufs=10))
        pssc = ctx.enter_context(tc.tile_pool(name="pssc", bufs=2, space="PSUM"))
        pso = ctx.enter_context(tc.tile_pool(name="pso", bufs=2, space="PSUM"))

        # ---- attention unit: score/exp group phase, then PV phase ----
        # The two phases alternate engines (ACT paces scores/exp, PE owns PV):
        # interleaving them measured WORSE (concurrent PE+ACT load trips the
        # HAM activity throttle to a 4/8 duty cycle); alternation keeps
        # instantaneous power lower. Normalize runs on DVE/GpSimd so the ACT
        # queue is pure exp and the next unit's scores are never delayed.
        def unit(p, qc, poA, poB):
            et = {}
            for hh in range(2):
                for quarter in range(4):
                    et[(hh, quarter)] = et_pool.tile(
                        [P, 4 * QW], bf16, name=f"et_{p}_{qc}_{hh}_{quarter}",
                        tag="et")
            for g in range(NG):
                half, goff = g // 2, (g % 2) * KPG * QW
                psA = pssc.tile([P, KPG * QW], f32,
                                name=f"scA_{p}_{qc}_{g}", tag="pssc")
                psB = pssc.tile([P, KPG * QW], f32,
                                name=f"scB_{p}_{qc}_{g}", tag="pssc")
                for j in range(KPG):
                    kc = g * KPG + j
                    mmr(psA[:, j * QW:(j + 1) * QW],
                        kt_t[p][0:64, kc * P:(kc + 1) * P],
                        qt_t[p][0:64, qc * QW:(qc + 1) * QW],
                        start=True, stop=True, tile_position=(0, 0))
                    mmr(psB[:, j * QW:(j + 1) * QW],
                        kt_t[p][64:128, kc * P:(kc + 1) * P],
                        qt_t[p][64:128, qc * QW:(qc + 1) * QW],
                        start=True, stop=True, tile_position=(64, 0))
                nc.scalar.activation(et[(0, half)][:, goff:goff + KPG * QW],
                                     psA[:], AF.Exp, scale=SCALE)
                nc.scalar.activation(et[(1, half)][:, goff:goff + KPG * QW],
                                     psB[:], AF.Exp, scale=SCALE)
            for kc in range(KC):
                half, koff = kc // 4, (kc % 4) * QW
                cA = (2 * p) * (HD + 1)
                cB = (2 * p + 1) * (HD + 1)
                mmr(poA[:], vpr_t[kc][:, cA:cA + HD + 1],
                    et[(0, half)][:, koff:koff + QW],
                    start=(kc == 0), stop=(kc == KC - 1))
                mmr(poB[:], vpr_t[kc][:, cB:cB + HD + 1],
                    et[(1, half)][:, koff:koff + QW],
                    start=(kc == 0), stop=(kc == KC - 1))
            ot_t = ot_pool.tile([P, QW], bf16, name=f"ot_{p}_{qc}", tag="ot")
            normalize_half(p, qc, poA, 0, ot_t)
            normalize_half(p, qc, poB, 1, ot_t)
            nc.vector.tensor_scalar_add(ot_t[:], ot_t[:], bv_t[p][:])
            return ot_t

        # 1/Z on ACT via Ln then Exp(-x): both live in the
        # natural_log_exp_and_others table set (no reloads against the
        # softmax Exp), and the chain runs in the ACT idle between a unit's
        # exp phase and the next unit's scores. The exact DVE reciprocal
        # (3.3us) measured slower here; the custom-DVE approx op returns
        # garbage on HW (uop table not honored by this runtime path).
        def normalize_half(p, qc, po, hh, ot_t):
            zl = zr_pool.tile([1, QW], f32, name=f"zl_{p}_{qc}_{hh}", tag="zr")
            nc.scalar.activation(zl[:], po[64:65, :], AF.Ln)
            zr = zr_pool.tile([1, QW], f32, name=f"zr_{p}_{qc}_{hh}", tag="zr")
            nc.scalar.activation(zr[:], zl[:], AF.Exp, scale=-1.0)
            zb = zb_pool.tile([64, QW], f32, name=f"zb_{p}_{qc}_{hh}", tag="zb")
            nc.gpsimd.partition_broadcast(zb[:], zr[:])
            nc.vector.tensor_mul(ot_t[hh * 64:(hh + 1) * 64, :],
                                 po[0:64, :], zb[:])

        def outproj(qc, ots):
            for tl in range(QW // P):
                tci = qc * (QW // P) + tl
                for dc in range(2):
                    ps = pp.tile([P, QW], f32, name=f"pout_{tci}_{dc}", tag="pp")
                    for pq in range(PAIRS):
                        mmr(ps[:], ots[pq][:, tl * P:(tl + 1) * P],
                            wo_t[(pq, dc)][:],
                            start=(pq == 0), stop=(pq == PAIRS - 1))
                    ost = os_pool.tile([P, QW], f32,
                                       name=f"os_{tci}_{dc}", tag="os")
                    nc.vector.tensor_copy(ost[:], ps[:])
                    nc.sync.dma_start(out[tci * P:(tci + 1) * P,
                                          dc * QW:(dc + 1) * QW], ost[:])

        # ---- Phase C: pair-outer sweep, next pair's Q/K proj interleaved ----
        ots_by_qc = {qc: [None] * PAIRS for qc in range(QCC)}
        for p in range(PAIRS):
            for qc in range(QCC):
                poA = pso.tile([HD + 1, QW], f32, name=f"poA_{p}_{qc}", tag="pso")
                poB = pso.tile([HD + 1, QW], f32, name=f"poB_{p}_{qc}", tag="pso")
                ots_by_qc[qc][p] = unit(p, qc, poA, poB)
                if p < PAIRS - 1:
                    qk_proj_quarter(p + 1, qc, pp)
                else:
                    outproj(qc, ots_by_qc[qc])
    nc.compile()
    return nc


def _get_nc():
    if "nc" not in _CACHE:
        _CACHE["nc"] = _build()
    return _CACHE["nc"]


def _in_maps(inputs):
    f = np.float32
    bf = ml_dtypes.bfloat16
    maps = []
    for c in range(NCORES):
        b, g = c // 2, c % 2
        hs = slice(g * HPC, (g + 1) * HPC)
        maps.append({
            "xqT": np.asarray(inputs["inputs_q"][b], f).T.astype(bf),
            "xkT": np.asarray(inputs["inputs_k"][b], f).T.astype(bf),
            "xvT": np.asarray(inputs["inputs_v"][b], f).T.astype(bf),
            "wq": np.asarray(inputs["Wq"], f)[:, hs, :].reshape(D, DH).astype(bf),
            "wk": np.asarray(inputs["Wk"], f)[:, hs, :].reshape(D, DH).astype(bf),
            "wv": np.asarray(inputs["Wv"], f)[:, hs, :].reshape(D, DH).astype(bf),
            "wo": np.asarray(inputs["Wo"], f)[hs].reshape(DH, D).astype(bf),
            "biases": np.stack(
                [np.asarray(inputs[nm], f)[hs].reshape(DH)[p * P:(p + 1) * P]
                 for nm in ("bq", "bk", "bv") for p in range(PAIRS)],
                axis=1).copy(),
        })
    return maps


def run_sharded(inputs, **kw):
    """Compile/run on all 8 cores; returns (full_output, BassKernelResults)."""
    from concourse.bass_utils import run_bass_kernel_spmd
    nc = _get_nc()
    res = run_bass_kernel_spmd(nc, _in_maps(inputs), core_ids=list(range(NCORES)), **kw)
    bo = np.asarray(inputs["bo"], np.float32)
    full = np.empty((B, S, D), np.float32)
    for b in range(B):
        full[b] = res.results[2 * b]["out"] + res.results[2 * b + 1]["out"] + bo
    return full, res


def kernel(**inputs) -> np.ndarray:
    full, _ = run_sharded(inputs)
    return full

